# revision 16
# baseline (speedup 1.0000x reference)
"""Trainium2 Bass kernel for nn_DeepResidualGCN (8-core SPMD).

Strategy (per conv, x15):
  z' = dis * (h @ W)  computed shard-local (PE transpose + matmul, bf16)
  AllGather z' shards -> z_full [N,128] bf16 in every core's HBM
  per dst-block of 128 nodes: dma_gather source rows (4 banks of <=32k rows,
  int16 idx, one gather call per (block,bank) cell capped at 1024 idxs/call
  due to the SWDGE descriptor-ring size, round-robined over 4 SWDGE queues
  for parallel Q7 descriptor generation), one-hot S generated on DVE
  (iota vs dst compare), scatter-add via PE matmuls S.T @ G accumulated in
  PSUM, epilogue applies dis[dst], residual adds and relu on ACT/DVE.
Self-loops are appended as ordinary edges; the symmetric norm
dis[src]*dis[dst] is factored into the z'-scale (src side) and the
PSUM-drain scale (dst side).
"""

import numpy as np
import ml_dtypes
from contextlib import ExitStack

from concourse import bass, bacc, mybir, tile
from concourse import bass_utils
from concourse.masks import make_identity

F32 = mybir.dt.float32
BF16 = mybir.dt.bfloat16
I16 = mybir.dt.int16

P = 128
GMAX = 8          # max chunks (of 128 idxs) per dma_gather call (ring limit)
NQ = 4            # SWDGE queues


def _build_schedule(L):
    """Sequence of (weight_index, kind) matching the reference layer loop."""
    convs = []
    for i in range(L):
        convs.append((i, "first0" if i == 0 else "first"))
        if i < L - 1:
            convs.append(((i - 1) % L, "second"))
    return convs


def _preprocess(x, edge_index, n_cores, bank_rows, group_blocks):
    """Host-side graph preprocessing. Returns per-core tables + common
    static structure (identical across cores so one SPMD program works)."""
    N = x.shape[0]
    src = edge_index[0].astype(np.int64)
    dst = edge_index[1].astype(np.int64)

    deg = np.bincount(dst, minlength=N).astype(np.float32) + 1.0
    dis = deg ** -0.5  # [N] f32

    # append self-loops
    src_all = np.concatenate([src, np.arange(N, dtype=np.int64)])
    dst_all = np.concatenate([dst, np.arange(N, dtype=np.int64)])

    NSH = N // n_cores
    T = -(-NSH // P)  # tiles (dst blocks) per core
    NBANK = -(-N // bank_rows)
    groups = [list(range(g, min(g + group_blocks, T)))
              for g in range(0, T, group_blocks)]

    # per-core sorted edge cells
    core_cells = []  # core -> dict[(block,bank)] -> (srel array, dloc array)
    for c in range(n_cores):
        m = (dst_all >= c * NSH) & (dst_all < (c + 1) * NSH)
        s = src_all[m]
        dl = (dst_all[m] - c * NSH).astype(np.int64)
        blk = dl >> 7
        bank = s // bank_rows
        srel = (s - bank * bank_rows).astype(np.int16)
        dloc = (dl & 127).astype(np.int16)
        order = np.lexsort((srel, bank, blk))
        s_, b_, k_, d_ = srel[order], blk[order], bank[order], dloc[order]
        cells = {}
        key = b_ * NBANK + k_
        bounds = np.flatnonzero(np.diff(key)) + 1
        starts = np.concatenate([[0], bounds])
        ends = np.concatenate([bounds, [len(key)]])
        for st, en in zip(starts, ends):
            cells[(int(b_[st]), int(k_[st]))] = (s_[st:en], d_[st:en])
        core_cells.append(cells)

    # common chunk counts per (block, bank): max over cores, ceil to chunks
    Kbb = np.zeros((T, NBANK), np.int64)
    for b in range(T):
        for k in range(NBANK):
            mx = max(len(core_cells[c].get((b, k), ((), ()))[0])
                     for c in range(n_cores))
            Kbb[b, k] = -(-mx // P) if mx > 0 else 0

    # chunk layout: block-major, bank-minor.  Per block: gather sub-calls
    # of <= GMAX chunks per (block, bank) cell, round-robin queues.
    blocks_info = []
    ch0 = 0
    qrr = 0
    for b in range(T):
        K_b = int(Kbb[b].sum())
        subs = []  # (bank, off_in_block, C, queue)
        off = 0
        for k in range(NBANK):
            left = int(Kbb[b, k])
            while left > 0:
                C = min(left, GMAX)
                subs.append((k, off, C, qrr % NQ))
                qrr += 1
                off += C
                left -= C
        assert off == K_b
        blocks_info.append({"ch0": ch0, "K": K_b, "subs": subs})
        ch0 += K_b
    NCH = ch0
    KBMAX = max(bi["K"] for bi in blocks_info)

    # per-core tables
    idx_tabs, dst_tabs, dis_tabs, x_locals = [], [], [], []
    for c in range(n_cores):
        idx_flat = np.zeros((NCH * P,), np.int16)
        dst_flat = np.full((NCH, P), -1, np.int16)
        for b in range(T):
            bi = blocks_info[b]
            pos = bi["ch0"]
            for k in range(NBANK):
                kch = int(Kbb[b, k])
                if kch == 0:
                    continue
                s_, d_ = core_cells[c].get((b, k), (np.zeros(0, np.int16),
                                                   np.zeros(0, np.int16)))
                n = len(s_)
                fl = idx_flat[pos * P: (pos + kch) * P]
                fl[:n] = s_
                df = dst_flat[pos: pos + kch].reshape(-1)
                df[:n] = d_
                pos += kch
        # wrap idx: global position i -> (i % 16, i // 16), replicated x8
        w = idx_flat.reshape(-1, 16).T  # [16, NCH*8]
        idx_tabs.append(np.tile(w, (8, 1)).copy())  # [128, NCH*8]
        dst_tabs.append(dst_flat.T.copy())  # [128, NCH]

        dl = np.zeros((T * P,), np.float32)
        dl[:NSH] = dis[c * NSH:(c + 1) * NSH]
        dis_tabs.append(dl.reshape(T, P).T.copy())  # [128, T]

        xl = np.zeros((T * P, x.shape[1]), np.float32)
        xl[:NSH] = x[c * NSH:(c + 1) * NSH]
        x_locals.append(xl)

    meta = dict(N=N, NSH=NSH, T=T, NBANK=NBANK, bank_rows=bank_rows,
                groups=groups, blocks_info=blocks_info, NCH=NCH, KBMAX=KBMAX)
    tabs = dict(idx=idx_tabs, dst=dst_tabs, dis=dis_tabs, x=x_locals)
    return meta, tabs


def _build_program(meta, L, has_bias, n_cores):
    NSH, T = meta["NSH"], meta["T"]
    NBANK, BANKR = meta["NBANK"], meta["bank_rows"]
    groups, blocks_info = meta["groups"], meta["blocks_info"]
    NCH, KBMAX = meta["NCH"], meta["KBMAX"]
    N = meta["N"]
    convs = _build_schedule(L)
    tail = NSH - P * (T - 1)  # rows in last tile

    nc = bacc.Bacc("TRN2", target_bir_lowering=False, debug=False,
                   enable_asserts=False, num_devices=n_cores,
                   num_swdge_queues=NQ)

    x_in = nc.dram_tensor("x_in", [T * P, P], F32, kind="ExternalInput").ap()
    idx_in = nc.dram_tensor("idx_in", [P, NCH * 8], I16, kind="ExternalInput").ap()
    dst_in = nc.dram_tensor("dst_in", [P, NCH], I16, kind="ExternalInput").ap()
    dis_in = nc.dram_tensor("dis_in", [P, T], F32, kind="ExternalInput").ap()
    ws_in = nc.dram_tensor("ws_in", [L, P, P], F32, kind="ExternalInput").ap()
    if has_bias:
        b_in = nc.dram_tensor("b_in", [L * P, P], F32, kind="ExternalInput").ap()
    h_out = nc.dram_tensor("h_out", [NSH, P], F32, kind="ExternalOutput").ap()

    zb = nc.dram_tensor("zbounce", [NSH, P], BF16, kind="Internal")
    zf = nc.dram_tensor("zfull", [N, P], BF16, kind="Internal")

    with tile.TileContext(nc) as tc:
        with ExitStack() as ctx:
            cst = ctx.enter_context(tc.tile_pool(name="cst", bufs=1))
            sb_hT = ctx.enter_context(tc.tile_pool(name="hT", bufs=3))
            sb_z = ctx.enter_context(tc.tile_pool(name="zt", bufs=3))
            sb_idx = ctx.enter_context(tc.tile_pool(name="idx", bufs=4))
            sb_dst = ctx.enter_context(tc.tile_pool(name="dst", bufs=4))
            sb_g = ctx.enter_context(tc.tile_pool(name="gth", bufs=3))
            sb_s = ctx.enter_context(tc.tile_pool(name="sel", bufs=3))
            sb_tmp = ctx.enter_context(tc.tile_pool(name="tmp", bufs=4))
            ps_tr = ctx.enter_context(tc.tile_pool(name="ptr", bufs=2, space="PSUM"))
            ps_z = ctx.enter_context(tc.tile_pool(name="pz", bufs=2, space="PSUM"))
            ps_agg = ctx.enter_context(tc.tile_pool(name="pagg", bufs=2, space="PSUM"))

            # ---- constants / persistent state ----
            ident = cst.tile([P, P], F32)
            make_identity(nc, ident[:])
            iota = cst.tile([P, KBMAX * P], I16)
            nc.gpsimd.iota(iota[:], pattern=[[0, KBMAX], [1, P]], base=0,
                           channel_multiplier=0)
            iota3 = iota[:].rearrange("p (c d) -> p c d", c=KBMAX)

            wf = cst.tile([P, L * P], F32)
            for i in range(L):
                nc.sync.dma_start(out=wf[:, i * P:(i + 1) * P], in_=ws_in[i])
            wb = cst.tile([P, L * P], BF16)
            nc.vector.tensor_copy(wb[:], wf[:])

            dis_sb = cst.tile([P, T], F32)
            nc.sync.dma_start(out=dis_sb[:], in_=dis_in[:])

            if has_bias:
                b_sb = cst.tile([P, L * P], F32)
                nc.sync.dma_start(
                    out=b_sb[:].rearrange("p (l f) -> l p f", l=L),
                    in_=b_in[:].rearrange("(l p) f -> l p f", p=P))

            h_sb = cst.tile([P, T * P], F32)
            nc.sync.dma_start(
                out=h_sb[:].rearrange("p (t f) -> p t f", f=P),
                in_=x_in[:].rearrange("(t p) f -> p t f", p=P))
            init_sb = cst.tile([P, T * P], F32)
            nc.vector.tensor_copy(init_sb[:], h_sb[:])

            zf_banks = [zf.ap()[k * BANKR: min((k + 1) * BANKR, N), :]
                        for k in range(NBANK)]

            # ---- conv sequence ----
            for ci, (wi, kind) in enumerate(convs):
                # transform: z' = dis * (h @ W[wi]) -> zbounce (bf16)
                for t in range(T):
                    rows = P if t < T - 1 else tail
                    trp = ps_tr.tile([P, P], F32, space="PSUM")
                    nc.tensor.transpose(trp[:], h_sb[:, t * P:(t + 1) * P],
                                        ident[:])
                    hT = sb_hT.tile([P, P], BF16)
                    nc.scalar.copy(hT[:], trp[:])
                    zp = ps_z.tile([P, P], F32, space="PSUM")
                    nc.tensor.matmul(zp[:], lhsT=hT[:],
                                     rhs=wb[:, wi * P:(wi + 1) * P],
                                     start=True, stop=True)
                    zt = sb_z.tile([P, P], BF16)
                    nc.scalar.mul(zt[:], zp[:], mul=dis_sb[:, t:t + 1])
                    nc.sync.dma_start(out=zb.ap()[t * P:t * P + rows, :],
                                      in_=zt[:rows, :])

                nc.gpsimd.collective_compute(
                    "AllGather", mybir.AluOpType.bypass,
                    replica_groups=[list(range(n_cores))],
                    ins=[zb.ap()], outs=[zf.ap()])

                # aggregate
                for g, blocks in enumerate(groups):
                    pg = ps_agg.tile([P, 4 * P], F32, space="PSUM")
                    for bi_, b in enumerate(blocks):
                        binfo = blocks_info[b]
                        K_b, ch0 = binfo["K"], binfo["ch0"]
                        idxt = sb_idx.tile([P, KBMAX * 8], I16)
                        nc.sync.dma_start(
                            out=idxt[:, :K_b * 8],
                            in_=idx_in[:, ch0 * 8:(ch0 + K_b) * 8])
                        dstt = sb_dst.tile([P, KBMAX], I16)
                        nc.sync.dma_start(
                            out=dstt[:, :K_b],
                            in_=dst_in[:, ch0:ch0 + K_b])
                        st = sb_s.tile([P, KBMAX, P], BF16)
                        nc.vector.tensor_tensor(
                            out=st[:, :K_b, :], in0=iota3[:, :K_b, :],
                            in1=dstt[:, :K_b].unsqueeze(2)
                                .to_broadcast((P, K_b, P)),
                            op=mybir.AluOpType.is_equal)
                        gt = sb_g.tile([P, KBMAX, P], BF16)
                        for (k, off, C, q) in binfo["subs"]:
                            nc.gpsimd.dma_gather(
                                out_ap=gt[:, off:off + C, :],
                                in_ap=zf_banks[k],
                                idxs_ap=idxt[:, off * 8:(off + C) * 8],
                                num_idxs=C * P, num_idxs_reg=C * P,
                                elem_size=P, queue_num=q)
                        for cc in range(K_b):
                            nc.tensor.matmul(
                                pg[:, bi_ * P:(bi_ + 1) * P],
                                lhsT=st[:, cc, :], rhs=gt[:, cc, :],
                                start=(cc == 0), stop=(cc == K_b - 1))

                    # epilogue per block
                    for bi_, b in enumerate(blocks):
                        hsl = h_sb[:, b * P:(b + 1) * P]
                        isl = init_sb[:, b * P:(b + 1) * P]
                        psl = pg[:, bi_ * P:(bi_ + 1) * P]
                        dsl = dis_sb[:, b:b + 1]
                        bsl = b_sb[:, wi * P:(wi + 1) * P] if has_bias else None
                        if kind == "first0":
                            if has_bias:
                                tmp = sb_tmp.tile([P, P], F32)
                                nc.scalar.mul(tmp[:], psl, mul=dsl)
                                nc.vector.tensor_add(tmp[:], tmp[:], bsl)
                                nc.scalar.activation(
                                    hsl, tmp[:],
                                    mybir.ActivationFunctionType.Relu)
                            else:
                                nc.scalar.activation(
                                    hsl, psl, mybir.ActivationFunctionType.Relu,
                                    scale=dsl)
                        elif kind == "first":
                            tmp = sb_tmp.tile([P, P], F32)
                            nc.scalar.mul(tmp[:], psl, mul=dsl)
                            if has_bias:
                                nc.vector.tensor_add(tmp[:], tmp[:], bsl)
                            nc.vector.tensor_add(isl, tmp[:], isl)
                            nc.scalar.activation(
                                hsl, isl, mybir.ActivationFunctionType.Relu)
                        else:  # second
                            tmp = sb_tmp.tile([P, P], F32)
                            nc.scalar.mul(tmp[:], psl, mul=dsl)
                            if has_bias:
                                nc.vector.tensor_add(tmp[:], tmp[:], bsl)
                            nc.vector.tensor_add(hsl, hsl, tmp[:])

            # ---- output ----
            nc.sync.dma_start(
                out=h_out[: (T - 1) * P, :].rearrange("(t p) f -> p t f", p=P),
                in_=h_sb[:, : (T - 1) * P].rearrange("p (t f) -> p t f", f=P))
            nc.sync.dma_start(out=h_out[(T - 1) * P: NSH, :],
                              in_=h_sb[:tail, (T - 1) * P: T * P])

    nc.compile()
    return nc


def _run(x, edge_index, Ws, bs, n_cores=8, bank_rows=25000, group_blocks=4,
         trace=False):
    x = np.asarray(x, np.float32)
    edge_index = np.asarray(edge_index)
    Ws = np.asarray(Ws, np.float32)
    bs = np.asarray(bs, np.float32)
    L = Ws.shape[0]
    N = x.shape[0]
    has_bias = bool(np.any(bs != 0))

    meta, tabs = _preprocess(x, edge_index, n_cores, bank_rows, group_blocks)
    nc = _build_program(meta, L, has_bias, n_cores)

    in_maps = []
    for c in range(n_cores):
        m = {
            "x_in": tabs["x"][c],
            "idx_in": tabs["idx"][c],
            "dst_in": tabs["dst"][c],
            "dis_in": tabs["dis"][c],
            "ws_in": Ws,
        }
        if has_bias:
            m["b_in"] = np.repeat(bs, P, axis=0).reshape(L * P, P).astype(np.float32)
        in_maps.append(m)

    res = bass_utils.run_bass_kernel_spmd(nc, in_maps,
                                          core_ids=list(range(n_cores)),
                                          trace=trace)
    out = np.concatenate([res.results[c]["h_out"] for c in range(n_cores)],
                         axis=0)
    if trace and res.exec_time_ns is not None:
        print("HW exec time:", res.exec_time_ns, "ns")
    return out[:N].astype(np.float32)


def kernel(x, edge_index, Ws, bs):
    return _run(x, edge_index, Ws, bs)
